# revision 13
# baseline (speedup 1.0000x reference)
"""Trainium2 Bass kernel for nn_Controller LSTM-policy loss (8 NeuronCores).

Math (see derivation in comments):
  - The reference scan carries (x, h, c) -> (h, h, c), so x == h for every
    step after the first: steps 1..7 use a single fused weight W_sum =
    W_ih + W_hh; step 0 uses W_ih alone (h=0).  Bias is b_ih + b_hh always.
  - The policy loss's random gather probs.flat[index] is folded on the host
    into a dense weight table w[a,o] = sum_b rewards[b,a']*[index==a*NOP+o]
    (scatter-add over the inputs), turning the loss into a dense reduction:
        loss_batch = sum_{a,o} w * (-logp),   logp = x - logZ_a
        sum_o -w*logp        = logZ*A4 - A3   (A4 = sum_o w, A3 = sum_o w*x)
        sum_o -p*logp        = logZ - A2/A1   (A1 = sum_o e^x, A2 = sum_o x e^x)
        loss = (loss_init + T1 + NB*REG*(T2 + sum W_ih[:F]^2)) / NB
  - Sharding: tensor-parallel over the hidden/gate dim (256 units/core) so
    the transposed weights stay resident in SBUF (data-parallel would
    re-stream 32MB of weights from HBM every step).  Each step runs two
    half AllGathers (bf16 [128,256] per rank, one per unit-half) so the
    first exchange overlaps the second half's matmuls and the next step's
    even-k-tile matmuls overlap the second exchange — keeping the PE busy
    through the exchange latency (HAM stays warm).  All loss statistics
    (sum e^x, sum x e^x, sum w x, sum w, using e^x = sig(x)/sig(-x) to stay
    in one act-table set) accumulate inline via ones-selector matmuls into
    PSUM; one small final AllReduce combines them across cores.
"""

import numpy as np
import ml_dtypes

# ---- problem constants (hardcoded per contract) ----
F = 256          # num_feature
H = 2048         # num_op / LSTM hidden
T = 8            # max_order
NB = 32          # num_batch
NA = F * T       # num_action = 2048
REG = 1e-3
NCORES = 8
UPC = H // NCORES    # 256 units per core
KT = H // 128        # 16 k-tiles

BF16 = ml_dtypes.bfloat16

_CACHE = {}


def _build_program():
    import concourse.bass as bass
    import concourse.tile as tile
    from concourse import bacc, mybir
    from concourse.bass import ds

    dt = mybir.dt
    AF = mybir.ActivationFunctionType
    ALU = mybir.AluOpType

    nc = bacc.Bacc("TRN2", target_bir_lowering=False, debug=False,
                   num_devices=NCORES)

    # ---- per-core external inputs ----
    wihT_d = nc.dram_tensor("wihT", [H, 4 * UPC], dt.bfloat16, kind="ExternalInput")
    wsumT_d = nc.dram_tensor("wsumT", [H, 4 * UPC], dt.bfloat16, kind="ExternalInput")
    x0T_d = nc.dram_tensor("x0T", [H, F], dt.bfloat16, kind="ExternalInput")
    bias_d = nc.dram_tensor("biasg", [128, 8], dt.float32, kind="ExternalInput")
    wt2_d = nc.dram_tensor("wt2", [UPC, T, 512], dt.float32, kind="ExternalInput")
    wreg_d = nc.dram_tensor("wreg", [F // NCORES, H], dt.float32, kind="ExternalInput")
    linit_d = nc.dram_tensor("linit", [1, 1], dt.float32, kind="ExternalInput")
    out_d = nc.dram_tensor("out", [1, 1], dt.float32, kind="ExternalOutput")

    with tile.TileContext(nc) as tc:
        with tc.tile_pool(name="wp", bufs=1) as wp, \
             tc.tile_pool(name="mvp", bufs=2) as mvp, \
             tc.tile_pool(name="acts", bufs=2) as acts, \
             tc.tile_pool(name="eexp", bufs=2) as eexp, \
             tc.tile_pool(name="dram", bufs=2, space="DRAM") as dram, \
             tc.tile_pool(name="ps", bufs=1, space="PSUM") as ps:

            # ---- resident tiles ----
            wih = wp.tile([128, KT, 4 * UPC], dt.bfloat16, tag="wih")
            wsum = wp.tile([128, KT, 4 * UPC], dt.bfloat16, tag="wsum")
            wt2 = wp.tile([128, 2, T, 512], dt.float32, tag="wt2")
            biasg = wp.tile([128, 8], dt.float32, tag="biasg")
            wreg = wp.tile([F // NCORES, H], dt.float32, tag="wreg")
            linit = wp.tile([1, 1], dt.float32, tag="linit")
            cstate = wp.tile([128, 512], dt.float32, tag="cstate")
            xstash = wp.tile([128, 2 * T, F], dt.float32, tag="xstash")
            z15 = wp.tile([128, 15], dt.float32, tag="z15")
            ones8 = wp.tile([8, 1], dt.float32, tag="ones8")
            ones32 = wp.tile([32, 1], dt.float32, tag="ones32")

            # ---- init DMAs needed for step 0 ----
            for g in range(8):
                nc.sync.dma_start(
                    out=wih[:, 2 * g:2 * (g + 1), :],
                    in_=wihT_d[:, :].rearrange("(kk p) m -> p kk m", p=128)[:, 2 * g:2 * (g + 1), :])
            # moving operand, split by k-tile parity (even tiles arrive via
            # AG#0 = unit-half 0 of every rank, odd via AG#1)
            mve = mvp.tile([128, KT // 2, F], dt.bfloat16, tag="mve")
            mvo = mvp.tile([128, KT // 2, F], dt.bfloat16, tag="mvo")
            for kk in range(KT):
                dst = mve if kk % 2 == 0 else mvo
                nc.sync.dma_start(out=dst[:, kk // 2, :],
                                  in_=x0T_d[kk * 128:(kk + 1) * 128, :])
            nc.sync.dma_start(out=biasg[:], in_=bias_d[:, :])

            nc.vector.memset(cstate[:], 0.0)
            nc.vector.memset(z15[:], 0.0)
            nc.vector.memset(z15[:, 7:8], 1.0)
            nc.vector.memset(ones8[:], 1.0)
            nc.vector.memset(ones32[:], 1.0)

            # lower-priority loads (needed from t>=1 / inline stats); emitted
            # after the step-0-critical DMAs so those win queue priority
            for g in range(8):
                nc.sync.dma_start(
                    out=wsum[:, 2 * g:2 * (g + 1), :],
                    in_=wsumT_d[:, :].rearrange("(kk p) m -> p kk m", p=128)[:, 2 * g:2 * (g + 1), :])
            for j in range(2):
                nc.sync.dma_start(
                    out=wt2[:, j, :, :],
                    in_=wt2_d[j * 128:(j + 1) * 128, :, :])
            nc.sync.dma_start(out=wreg[:], in_=wreg_d[:, :])
            nc.sync.dma_start(out=linit[:], in_=linit_d[:, :])

            # ---- LSTM scan ----
            # a23 (sum w*x, sum w) and a01 (sum e^x, sum x e^x) accumulate
            # inline across all (t, uh); e^x = sigmoid(x)/sigmoid(-x) keeps
            # the whole kernel inside one activation-table set.
            a23 = ps.tile([8, 512], dt.float32, tag="a23")
            a01 = ps.tile([8, 512], dt.float32, tag="a01")
            a23_first = True
            for t in range(T):
                wt = wih if t == 0 else wsum
                banks = [ps.tile([128, 512], dt.float32, tag=f"bank{g}",
                                 name=f"bank{g}_t{t}")
                         for g in range(4)]
                # gate matmuls: finish unit-half 0 completely first (even
                # k-tiles arrive via prev AG#0, odd via prev AG#1) so its
                # h-exchange is issued at the midpoint of the step's PE work.
                for uh in range(2):
                    sl = ds(uh * 256, 256)
                    for par in range(2):
                        src = mve if par == 0 else mvo
                        for g in range(4):
                            m = g * 2 + uh
                            for j in range(KT // 2):
                                nc.tensor.matmul(
                                    banks[g][:, sl],
                                    lhsT=wt[:, 2 * j + par, m * 128:(m + 1) * 128],
                                    rhs=src[:, j, :],
                                    start=(par == 0 and j == 0),
                                    stop=(par == 1 and j == KT // 2 - 1))
                if t < T - 1:
                    mve = mvp.tile([128, KT // 2, F], dt.bfloat16, tag="mve")
                    mvo = mvp.tile([128, KT // 2, F], dt.bfloat16, tag="mvo")
                for uh in range(2):
                    sl = ds(uh * 256, 256)
                    # activations for this unit-half
                    si = acts.tile([128, F], dt.float32, tag="si")
                    sf = acts.tile([128, F], dt.float32, tag="sf")
                    tg = acts.tile([128, F], dt.float32, tag="tg")
                    so = acts.tile([128, F], dt.float32, tag="so")
                    tmp = acts.tile([128, F], dt.float32, tag="tmp")
                    tcn = acts.tile([128, F], dt.float32, tag="tcn")
                    bia = lambda g_: biasg[:, ds(g_ * 2 + uh, 1)]
                    nc.scalar.activation(si[:], banks[0][:, sl], AF.Sigmoid, bias=bia(0))
                    nc.scalar.activation(sf[:], banks[1][:, sl], AF.Sigmoid, bias=bia(1))
                    nc.scalar.activation(tg[:], banks[2][:, sl], AF.Tanh, bias=bia(2))
                    nc.scalar.activation(so[:], banks[3][:, sl], AF.Sigmoid, bias=bia(3))
                    cs = cstate[:, sl]
                    nc.vector.tensor_mul(tmp[:], si[:], tg[:])
                    nc.vector.tensor_mul(cs, sf[:], cs)
                    nc.vector.tensor_add(cs, cs, tmp[:])
                    nc.scalar.activation(tcn[:], cs, AF.Tanh)
                    xsl = xstash[:, t * 2 + uh, :]
                    nc.vector.tensor_mul(xsl, so[:], tcn[:])
                    if t < T - 1:
                        # per-half AllGather so AG#0 overlaps uh1's matmuls
                        hb = acts.tile([128, F], dt.bfloat16, tag="hb")
                        nc.vector.tensor_mul(hb[:], so[:], tcn[:])
                        agin = dram.tile([128, F], dt.bfloat16, tag=f"agin{uh}",
                                         name=f"agin{uh}_t{t}")
                        agout = dram.tile([H // 2, F], dt.bfloat16, tag=f"agout{uh}",
                                          name=f"agout{uh}_t{t}")
                        nc.sync.dma_start(out=agin[:, :], in_=hb[:])
                        nc.gpsimd.collective_compute(
                            "AllGather", ALU.bypass,
                            replica_groups=[list(range(NCORES))],
                            ins=[agin[:].opt()], outs=[agout[:].opt()])
                        dst = mve if uh == 0 else mvo
                        for g in range(4):
                            nc.sync.dma_start(
                                out=dst[:, 2 * g:2 * (g + 1), :],
                                in_=agout[:, :].rearrange("(j p) f -> p j f", p=128)[:, 2 * g:2 * (g + 1), :])
                    # inline loss stats: WX = w*x overwrites wt2[...,0:256];
                    # E = sigmoid(x)/sigmoid(-x), EX = E*x; the stat matmuls
                    # double as PE filler during the AllGather waits
                    nc.vector.tensor_mul(wt2[:, uh, t, 0:256], wt2[:, uh, t, 256:512], xsl)
                    eex = eexp.tile([128, 512], dt.float32, tag="eex")
                    sneg = eexp.tile([128, 256], dt.float32, tag="sneg")
                    nc.scalar.activation(eex[:, 0:256], xsl, AF.Sigmoid)
                    nc.scalar.activation(sneg[:], xsl, AF.Sigmoid, scale=-1.0)
                    nc.vector.reciprocal(sneg[:], sneg[:])
                    nc.vector.tensor_mul(eex[:, 0:256], eex[:, 0:256], sneg[:])
                    nc.vector.tensor_mul(eex[:, 256:512], eex[:, 0:256], xsl)
                    last = (t == T - 1 and uh == 1)
                    nc.tensor.matmul(a23[:, :], lhsT=z15[:, ds(7 - t, 8)],
                                     rhs=wt2[:, uh, t, :], start=a23_first, stop=last)
                    nc.tensor.matmul(a01[:, :], lhsT=z15[:, ds(7 - t, 8)],
                                     rhs=eex[:, :], start=a23_first, stop=last)
                    a23_first = False

            # ---- regularizer partial: sum of squares of this core's W_ih[:F] rows
            sqs = wp.tile([F // NCORES, H], dt.float32, tag="sqs")
            sqsum = wp.tile([F // NCORES, 1], dt.float32, tag="sqsum")
            nc.vector.scalar_tensor_tensor(
                out=sqs[:], in0=wreg[:], scalar=1.0, in1=wreg[:],
                op0=ALU.mult, op1=ALU.mult, accum_out=sqsum[:])
            sregp = ps.tile([1, 1], dt.float32, tag="sregp")
            nc.tensor.matmul(sregp[:, :], lhsT=ones32[:, 0:1], rhs=sqsum[:, :],
                             start=True, stop=True)

            # ---- AllReduce partial stats across cores ----
            acopy = wp.tile([8, 1024], dt.float32, tag="acopy")
            nc.vector.tensor_copy(acopy[:, 0:512], a01[:, :])
            nc.vector.tensor_copy(acopy[:, 512:1024], a23[:, :])
            sregrow = wp.tile([1, 1024], dt.float32, tag="sregrow")
            nc.vector.memset(sregrow[:], 0.0)
            nc.vector.tensor_copy(sregrow[:, 0:1], sregp[:, :])
            arin = dram.tile([9, 1024], dt.float32, tag="arin")
            arout = dram.tile([9, 1024], dt.float32, tag="arout")
            nc.sync.dma_start(out=arin[0:8, :], in_=acopy[:])
            nc.sync.dma_start(out=arin[8:9, :], in_=sregrow[:])
            nc.gpsimd.collective_compute(
                "AllReduce", ALU.add,
                replica_groups=[list(range(NCORES))],
                ins=[arin[:].opt()], outs=[arout[:].opt()])
            apost = wp.tile([8, 1024], dt.float32, tag="apost")
            sregpost = wp.tile([1, 1], dt.float32, tag="sregpost")
            nc.sync.dma_start(out=apost[:], in_=arout[0:8, :])
            nc.sync.dma_start(out=sregpost[:], in_=arout[8:9, 0:1])

            # ---- final scalar ----
            logz = wp.tile([8, 256], dt.float32, tag="logz")
            rec = wp.tile([8, 256], dt.float32, tag="rec")
            pm = wp.tile([8, 256], dt.float32, tag="pm")
            ent = wp.tile([8, 256], dt.float32, tag="ent")
            t1 = wp.tile([8, 256], dt.float32, tag="t1")
            contrib = wp.tile([8, 256], dt.float32, tag="contrib")
            csum = wp.tile([8, 1], dt.float32, tag="csum")
            A1 = apost[:, 0:256]
            A2 = apost[:, 256:512]
            A3 = apost[:, 512:768]
            A4 = apost[:, 768:1024]
            nc.scalar.activation(logz[:], A1, AF.Ln)
            nc.vector.reciprocal(rec[:], A1)
            nc.vector.tensor_mul(pm[:], rec[:], A2)
            nc.vector.tensor_sub(ent[:], logz[:], pm[:])
            nc.vector.tensor_mul(t1[:], logz[:], A4)
            nc.vector.tensor_sub(t1[:], t1[:], A3)
            nc.vector.scalar_tensor_tensor(
                out=contrib[:], in0=ent[:], scalar=float(NB * REG),
                in1=t1[:], op0=ALU.mult, op1=ALU.add, accum_out=csum[:])
            finp = ps.tile([1, 1], dt.float32, tag="finp")
            nc.tensor.matmul(finp[:, :], lhsT=ones8[:, 0:1], rhs=csum[:, :],
                             start=True, stop=True)
            tot = wp.tile([1, 1], dt.float32, tag="tot")
            f1 = wp.tile([1, 1], dt.float32, tag="f1")
            fout = wp.tile([1, 1], dt.float32, tag="fout")
            nc.vector.tensor_copy(tot[:], finp[:, :])
            nc.vector.scalar_tensor_tensor(
                out=f1[:], in0=sregpost[:], scalar=float(NB * REG),
                in1=tot[:], op0=ALU.mult, op1=ALU.add)
            nc.vector.tensor_add(f1[:], f1[:], linit[:])
            nc.vector.tensor_scalar_mul(fout[:], f1[:], 1.0 / NB)
            nc.sync.dma_start(out=out_d[:, :], in_=fout[:])

    nc.compile()
    return nc


def _prep_in_maps(actions, rewards, loss_init, input0, W_ih, W_hh, b_ih, b_hh, index):
    W_ih = np.asarray(W_ih, dtype=np.float32)
    W_hh = np.asarray(W_hh, dtype=np.float32)
    b = np.asarray(b_ih, dtype=np.float32) + np.asarray(b_hh, dtype=np.float32)
    input0 = np.asarray(input0, dtype=np.float32)
    rewards = np.asarray(rewards, dtype=np.float32)
    index = np.asarray(index).astype(np.int64)
    loss_init = np.asarray(loss_init, dtype=np.float32)

    Wsum = W_ih + W_hh
    x0T = np.ascontiguousarray(input0.T).astype(BF16)     # [H, F]

    w = np.bincount(index.ravel(),
                    weights=rewards.ravel().astype(np.float64),
                    minlength=NA * H).astype(np.float32).reshape(NA, H)

    in_maps = []
    for k in range(NCORES):
        J = np.arange(UPC * k, UPC * (k + 1))
        rows = np.concatenate([J, H + J, 2 * H + J, 3 * H + J])
        wihT = np.ascontiguousarray(W_ih[rows].T).astype(BF16)    # [H, 1024]
        wsumT = np.ascontiguousarray(Wsum[rows].T).astype(BF16)   # [H, 1024]
        biasg = np.ascontiguousarray(b[rows].reshape(8, 128).T).astype(np.float32)
        wslice = w[:, UPC * k:UPC * (k + 1)].T                    # [op, action]
        wtf = wslice.reshape(UPC, F, T).transpose(0, 2, 1)        # [op, t, f]
        wt2 = np.zeros((UPC, T, 512), dtype=np.float32)
        wt2[:, :, 256:512] = wtf
        wreg = np.ascontiguousarray(
            W_ih[(F // NCORES) * k:(F // NCORES) * (k + 1), :]).astype(np.float32)
        in_maps.append({
            "wihT": wihT,
            "wsumT": wsumT,
            "x0T": x0T,
            "biasg": biasg,
            "wt2": wt2,
            "wreg": wreg,
            "linit": loss_init.reshape(1, 1),
        })
    return in_maps


def kernel(**inputs) -> np.ndarray:
    from concourse.bass_utils import run_bass_kernel_spmd

    if "nc" not in _CACHE:
        _CACHE["nc"] = _build_program()
    nc = _CACHE["nc"]

    in_maps = _prep_in_maps(**inputs)
    try:
        res = run_bass_kernel_spmd(nc, in_maps, core_ids=list(range(NCORES)),
                                   **_CACHE.get("run_kwargs", {}))
    except Exception:
        # one retry: transient NRT_EXEC_UNIT_UNRECOVERABLE wedges clear on rerun
        res = run_bass_kernel_spmd(nc, in_maps, core_ids=list(range(NCORES)),
                                   **_CACHE.get("run_kwargs", {}))
    _CACHE["last_results"] = res
    out = np.asarray(res.results[0]["out"], dtype=np.float32).reshape(1)
    return out
